# revision 72
# baseline (speedup 1.0000x reference)
"""Multi-head attention (B=4, S=2048, D=1024, H=16) on 8 TRN2 NeuronCores.

Data-parallel over the 64 (batch, head) attention pairs: 8 pairs per core.
Per pair, on-device (all matmul inputs bf16, PSUM accumulation fp32):
  q^T = [Wq.T; bq].T @ [X_q^T; 1]           -> [64, 2048]
  k^T = [Wk.T; bk].T @ [X_k^T; 1]           -> [64, 2048]
  v'  = [X_v^T; 1].T @ [[Wv.T, 0]; [bv, 1]] -> [2048, 65]  (ones column)
  S^T[ki, qi] = k^T.T @ q^T  (contraction over head dim 64)
  P^T = exp(S^T / 8)   -- split between ScalarE (exact spline exp) and
                          VectorE (Schraudolph bf16-bit exp) so neither
                          engine gates the PE
  out'[d', qi] = v'.T @ P^T                 -> [65, 2048]
Row 64 of out' is the softmax denominator (via the ones column of v');
the host divides and reassembles. exp needs no max subtraction: scores/8
has stddev ~0.33 for these inputs, far from fp32 overflow.

Attention runs as four passes over qi-quarters (512 columns). Per
ki-chunk-PAIR one row-tile-packed scores slot computes both chunks
concurrently (chunk 2cp on PE array rows 0:63, chunk 2cp+1 on rows
64:127 — the stationaries come from [w|w]-doubled projection weights,
so rows 64:127 of qT/kT are real duplicated data, no padding). Each
chunk then gets one whole-chunk FD=512 exp — chunks alternate between
ScalarE (exact spline exp) and VectorE (Schraudolph) with their own
tiles — and one PV matmul into a 1-bank accumulator. Hard-won TRN2
scheduling rules baked in:
  - PV trails scores by FIVE chunks (pinned with add_dep_helper; sc
    bufs=7 one-bank tiles) so the in-order PE rides out exp-engine
    queueing jitter;
  - each PSUM/SBUF tile has exactly one writer and one reader engine —
    sharing a tile across engines serializes them on a false dep;
  - the PV stationaries keep full 128 partitions (vS garbage columns
    land in unread PV output rows) so LDWEIGHTS hides in the PE's
    background weight buffer;
  - the next pair's projections are woven in 8 installments through
    the current pair's attention so no engine sees a setup burst.
The kernel is exp-engine-bound (ScalarE ~86% busy, VectorE ~83%); the
PE runs warm at 2.4 GHz with ~73% occupancy.
"""

import numpy as np
import ml_dtypes

B, S, D, H = 4, 2048, 1024, 16
HD = D // H  # 64
N_CORES = 8
PAIRS_PER_CORE = (B * H) // N_CORES  # 8
KC = S // 128  # 16 ki chunks of 128
NQ = 4         # qi quarters of 512
BF16 = ml_dtypes.bfloat16

# exp split within each 1024-wide scores tile: ScalarE does [0, 512)
# into its own pT tile, VectorE does [512, 1024) into another. Separate
# tiles (and disjoint PSUM banks) keep the two exp engines fully
# independent — sharing one tile serializes them on a false WAW dep.
# Schraudolph constants for bf16-bit exp(s/8): bits = s*A + B -> int16
SCH_A = 16 * 1.4426950408889634  # 128*log2(e)/8
SCH_B = 16256.0 - 5.5 - 3.0      # bias centered so rel err ~ +-1.7%

_COMPILED = {}


def _build_nc():
    import concourse.bass as bass  # noqa: F401
    import concourse.mybir as mybir
    import concourse.tile as tile
    from concourse import bacc
    from concourse.tile_rust import add_dep_helper

    f32 = mybir.dt.float32
    bf16 = mybir.dt.bfloat16
    i16 = mybir.dt.int16

    nc = bacc.Bacc("TRN2", num_devices=N_CORES)
    xq = nc.declare_dram_parameter("xq", [PAIRS_PER_CORE, HD + 1, S], bf16, isOutput=False)
    xk = nc.declare_dram_parameter("xk", [PAIRS_PER_CORE, HD + 1, S], bf16, isOutput=False)
    xv = nc.declare_dram_parameter("xv", [PAIRS_PER_CORE, HD + 1, S], bf16, isOutput=False)
    wq = nc.declare_dram_parameter("wq", [HD + 1, 2 * HD], bf16, isOutput=False)
    wk = nc.declare_dram_parameter("wk", [HD + 1, 2 * HD], bf16, isOutput=False)
    wv = nc.declare_dram_parameter("wv", [HD + 1, HD + 1], bf16, isOutput=False)
    out = nc.declare_dram_parameter("out", [PAIRS_PER_CORE, HD + 1, S], bf16, isOutput=True)

    EXP = mybir.ActivationFunctionType.Exp
    MULT = mybir.AluOpType.mult
    ADD = mybir.AluOpType.add

    with tile.TileContext(nc) as tc:
        with (
            tc.tile_pool(name="consts", bufs=1) as consts,
            tc.tile_pool(name="ins", bufs=2) as ins_pool,
            tc.tile_pool(name="qk", bufs=2) as qk_pool,
            tc.tile_pool(name="vp", bufs=2) as v_pool,
            tc.tile_pool(name="pt", bufs=8) as pt_pool,
            tc.tile_pool(name="ob", bufs=4) as out_pool,
            tc.tile_pool(name="sc", bufs=7, space="PSUM") as sc_pool,
            tc.tile_pool(name="pv", bufs=1, space="PSUM") as pv_pool,
        ):
            w_q = consts.tile([HD + 1, 2 * HD], bf16)
            nc.sync.dma_start(out=w_q[:], in_=wq[:])
            w_k = consts.tile([HD + 1, 2 * HD], bf16)
            nc.sync.dma_start(out=w_k[:], in_=wk[:])
            w_v = consts.tile([HD + 1, HD + 1], bf16)
            nc.sync.dma_start(out=w_v[:], in_=wv[:])

            def proj_steps(j):
                """Returns ((qT, kT, vS), step_generator): DMAs pair j's
                inputs and allocates its SBUF tiles immediately; the
                generator emits the 8 projection installments (4 q/k
                blocks + 4 v groups) so the caller can spread them
                between attention chunks."""
                Xq = ins_pool.tile([HD + 1, S], bf16, tag="Xq", name="Xq")
                nc.sync.dma_start(out=Xq[:], in_=xq[j])
                Xk = ins_pool.tile([HD + 1, S], bf16, tag="Xk", name="Xk")
                nc.sync.dma_start(out=Xk[:], in_=xk[j])
                Xv = ins_pool.tile([HD + 1, S], bf16, tag="Xv", name="Xv")
                nc.sync.dma_start(out=Xv[:], in_=xv[j])

                # q^T, k^T projections -> [128, 2048] bf16 tiles. Full-128
                # stationary/rhs shapes let the PE hide LDWEIGHTS in its
                # background weight buffer (half-shape stationaries expose
                # ~90ns per weight switch). kT rows 64:128 are zeroed so the
                # padded contraction contributes nothing; qT's are zeroed so
                # no Inf/NaN garbage meets the 0*x products.
                qT = qk_pool.tile([128, S], bf16, tag="qT", name="qT")
                kT = qk_pool.tile([128, S], bf16, tag="kT", name="kT")
                vS = v_pool.tile([128, KC * 128], bf16, tag="vS", name="vS")

                def steps():
                  for n4 in range(4):
                    ps_q = sc_pool.tile([128, 512], f32, tag="sca", name="ps_q")
                    ps_k = sc_pool.tile([128, 512], f32, tag="sca", name="ps_k")
                    col = n4 * 512
                    nc.tensor.matmul(ps_q[:], w_q[:], Xq[:, col : col + 512],
                                     start=True, stop=True)
                    nc.tensor.matmul(ps_k[:], w_k[:], Xk[:, col : col + 512],
                                     start=True, stop=True)
                    # all PSUM->SBUF casts live on VectorE: its 2x-packed
                    # bf16 cast (~392 ns) beats ScalarE's copy (~570 ns), so
                    # ScalarE spends the freed time on extra exp chunks
                    nc.vector.tensor_copy(qT[:, col : col + 512], ps_q[:])
                    nc.vector.tensor_copy(kT[:, col : col + 512], ps_k[:])
                    yield

                  # v' projection: [2048, 65] bf16, chunk c at columns c*128
                  # (columns 65:128 of each chunk are garbage; they only
                  # feed PV output rows 65:128, which are never read)
                  vS3 = vS.rearrange("p (c d) -> p c d", d=128)
                  for g in range(4):
                    ps_v = sc_pool.tile(
                        [128, 4 * (HD + 1)], f32,
                        tag="sca", name="ps_v",
                    )
                    for c4 in range(4):
                        c = g * 4 + c4
                        nc.tensor.matmul(
                            ps_v[:, c4 * (HD + 1) : (c4 + 1) * (HD + 1)],
                            Xv[:, c * 128 : (c + 1) * 128], w_v[:],
                            start=True, stop=True,
                        )
                    nc.vector.tensor_copy(
                        vS3[:, g * 4 : (g + 1) * 4, 0 : HD + 1],
                        ps_v[:].rearrange("p (c d) -> p c d", d=HD + 1),
                    )
                    if g < 3:
                        yield

                return (qT, kT, vS), steps()

            def run_proj(pair):
                """Drain a proj_steps generator, returning the tiles."""
                state, gen = pair
                for _ in gen:
                    pass
                return state

            TRAIL = 5

            def emit_attention_pass(j, h2, qT, kT, vS, interleave=None):
                # two qi-quarter sub-passes per call: per chunk one scores
                # matmul into a 1-bank [128,512] tile (bufs=6 -> a FOUR
                # chunk trail), one whole-chunk exp on a single engine
                # (chunks alternate ScalarE / VectorE so each tile has one
                # reader and one writer), and one PV matmul into a
                # double-buffered 1-bank accumulator.
                for q4 in (2 * h2, 2 * h2 + 1):
                    base = q4 * 512
                    pv = pv_pool.tile([128, 512], f32, tag="pv", name="pv")

                    def emit_scores_exp_pair(cp):
                        # row-tiled pack: chunk 2cp on array rows 0:63,
                        # chunk 2cp+1 on rows 64:127 — both K=64 matmuls
                        # stream their N=512 columns concurrently
                        c0, c1 = 2 * cp, 2 * cp + 1
                        sca = sc_pool.tile([128, 512], f32, tag="sca", name="sca")
                        scb = sc_pool.tile([128, 512], f32, tag="sca", name="scb")
                        nc.tensor.matmul(
                            sca[:], kT[0:HD, c0 * 128 : (c0 + 1) * 128],
                            qT[0:HD, base : base + 512],
                            start=True, stop=True,
                        )
                        mm = nc.tensor.matmul(
                            scb[:], kT[HD:128, c1 * 128 : (c1 + 1) * 128],
                            qT[HD:128, base : base + 512],
                            start=True, stop=True,
                        )
                        def exp_into(pT, sc_t, gc):
                            # 37 of every 64 chunks on ScalarE (exact exp),
                            # the rest on VectorE Schraudolph — the ratio
                            # that levels both engines once VectorE owns
                            # all the casts
                            if (gc * 37) // 64 != ((gc - 1) * 37) // 64:
                                nc.scalar.activation(
                                    pT[:], sc_t[:], EXP, scale=0.125
                                )
                            else:
                                nc.vector.tensor_scalar(
                                    pT[:].bitcast(i16), sc_t[:],
                                    SCH_A, SCH_B, MULT, ADD,
                                )

                        gq = (base // 512) * KC
                        pTa = pt_pool.tile([128, 512], bf16, tag="pTa", name="pTa")
                        exp_into(pTa, sca, gq + c0)
                        pTb = pt_pool.tile([128, 512], bf16, tag="pTa", name="pTb")
                        exp_into(pTb, scb, gq + c1)
                        return {c0: (pTa, mm), c1: (pTb, mm)}

                    def emit_pv(c, pT, after_mm):
                        mm = nc.tensor.matmul(
                            pv[:], vS[:, c * 128 : (c + 1) * 128], pT[:],
                            start=(c == 0), stop=(c == KC - 1),
                        )
                        if after_mm is not None:
                            add_dep_helper(
                                mm.ins, after_mm.ins, sync=False,
                                reason="pv trails scores",
                            )

                    pend = {}
                    for cp in range((TRAIL + 1) // 2):
                        pend.update(emit_scores_exp_pair(cp))
                    for c in range(KC):
                        nxt = c + TRAIL
                        if nxt < KC and nxt % 2 == 0 and nxt // 2 >= (TRAIL + 1) // 2:
                            pend.update(emit_scores_exp_pair(nxt // 2))
                        elif c % 2 == 1 and c + TRAIL + 1 < KC and (c + TRAIL + 1) // 2 >= (TRAIL + 1) // 2:
                            pend.update(emit_scores_exp_pair((c + TRAIL + 1) // 2))
                        pT_c, _ = pend.pop(c)
                        after = pend[c + TRAIL][1] if c + TRAIL in pend else None
                        emit_pv(c, pT_c, after)
                        if interleave is not None and c % 8 == 5:
                            # weave one installment of the next pair's
                            # projections between chunk groups
                            try:
                                next(interleave)
                            except StopIteration:
                                interleave = None
                    ob = out_pool.tile([HD + 1, 512], bf16, tag="ob", name="ob")
                    nc.vector.tensor_copy(ob[:], pv[0 : HD + 1, :])
                    nc.sync.dma_start(
                        out=out[j, :, base : base + 512], in_=ob[:]
                    )

            # pipeline pairs: pair j+1's DMA + projections are woven in
            # 8 installments through pair j's second attention pass, so
            # neither the PE nor the exp engines see a projection burst
            # at pair boundaries.
            state = run_proj(proj_steps(0))
            for j in range(PAIRS_PER_CORE):
                if j + 1 < PAIRS_PER_CORE:
                    nxt_state, gen = proj_steps(j + 1)
                    emit_attention_pass(j, 0, *state, interleave=gen)
                    emit_attention_pass(j, 1, *state, interleave=gen)
                    for _ in gen:
                        pass
                    state = nxt_state
                else:
                    emit_attention_pass(j, 0, *state)
                    emit_attention_pass(j, 1, *state)
    nc.finalize()
    return nc


def _get_nc():
    if "nc" not in _COMPILED:
        _COMPILED["nc"] = _build_nc()
    return _COMPILED["nc"]


def _prep_inputs(query, key_, value, Wq, bq, Wk, bk, Wv, bv):
    """Host-side repack: per (b,h) pair, [65, 2048] bf16 transposed-augmented."""
    def to_pairs(x):
        # [B, S, D] -> [B*H, HD, S] with ones row appended -> [B*H, HD+1, S]
        x = np.asarray(x, dtype=np.float32)
        x = x.reshape(B, S, H, HD).transpose(0, 2, 3, 1).reshape(B * H, HD, S)
        ones = np.ones((B * H, 1, S), dtype=np.float32)
        return np.ascontiguousarray(
            np.concatenate([x, ones], axis=1).astype(BF16)
        )

    xq_all = to_pairs(query)
    xk_all = to_pairs(key_)
    xv_all = to_pairs(value)

    Wq = np.asarray(Wq, np.float32); bq = np.asarray(bq, np.float32)
    Wk = np.asarray(Wk, np.float32); bk = np.asarray(bk, np.float32)
    Wv = np.asarray(Wv, np.float32); bv = np.asarray(bv, np.float32)
    wq_aug = np.concatenate([Wq.T, bq[None, :]], axis=0)
    wq_aug = np.concatenate([wq_aug, wq_aug], axis=1).astype(BF16)
    wk_aug = np.concatenate([Wk.T, bk[None, :]], axis=0)
    wk_aug = np.concatenate([wk_aug, wk_aug], axis=1).astype(BF16)
    wv_aug = np.zeros((HD + 1, HD + 1), np.float32)
    wv_aug[:HD, :HD] = Wv.T
    wv_aug[HD, :HD] = bv
    wv_aug[HD, HD] = 1.0
    wv_aug = wv_aug.astype(BF16)

    in_maps = []
    for i in range(N_CORES):
        sl = slice(i * PAIRS_PER_CORE, (i + 1) * PAIRS_PER_CORE)
        in_maps.append({
            "xq": np.ascontiguousarray(xq_all[sl]),
            "xk": np.ascontiguousarray(xk_all[sl]),
            "xv": np.ascontiguousarray(xv_all[sl]),
            "wq": wq_aug, "wk": wk_aug, "wv": wv_aug,
        })
    return in_maps


def _postprocess(outs):
    """outs: list of 8 arrays [8, 65, 2048] -> [B, S, D] float32."""
    full = np.concatenate(outs, axis=0).astype(np.float32)  # [64, 65, 2048]
    num = full[:, :HD, :]                # [64, 64, 2048]  (x_att^T unnormalized)
    den = full[:, HD : HD + 1, :]        # [64, 1, 2048]
    att = num / den                      # [B*H, HD, S]
    att = att.reshape(B, H, HD, S).transpose(0, 3, 1, 2).reshape(B, S, D)
    return np.ascontiguousarray(att.astype(np.float32))


def kernel(query, key_, value, Wq, bq, Wk, bk, Wv, bv, _trace=False, _res_box=None):
    import time

    from concourse.bass_utils import run_bass_kernel_spmd

    nc = _get_nc()
    in_maps = _prep_inputs(query, key_, value, Wq, bq, Wk, bk, Wv, bv)
    last_err = None
    for attempt in range(3):
        try:
            res = run_bass_kernel_spmd(
                nc, in_maps, core_ids=list(range(N_CORES)), trace=_trace
            )
            outs = [np.asarray(res.results[i]["out"]) for i in range(N_CORES)]
            break
        except Exception as e:  # transient device teardown races
            last_err = e
            time.sleep(3.0)
    else:
        raise last_err
    if _res_box is not None:
        _res_box.append(res)
    return _postprocess(outs)


# revision 73
# speedup vs baseline: 1.0462x; 1.0462x over previous
"""Multi-head attention (B=4, S=2048, D=1024, H=16) on 8 TRN2 NeuronCores.

Data-parallel over the 64 (batch, head) attention pairs: 8 pairs per core.
Per pair, on-device (all matmul inputs bf16, PSUM accumulation fp32):
  q^T = [Wq.T; bq].T @ [X_q^T; 1]           -> [64, 2048]
  k^T = [Wk.T; bk].T @ [X_k^T; 1]           -> [64, 2048]
  v'  = [X_v^T; 1].T @ [[Wv.T, 0]; [bv, 1]] -> [2048, 65]  (ones column)
  S^T[ki, qi] = k^T.T @ q^T  (contraction over head dim 64)
  P^T = exp(S^T / 8)   -- split between ScalarE (exact spline exp) and
                          VectorE (Schraudolph bf16-bit exp) so neither
                          engine gates the PE
  out'[d', qi] = v'.T @ P^T                 -> [65, 2048]
Row 64 of out' is the softmax denominator (via the ones column of v');
the host divides and reassembles. exp needs no max subtraction: scores/8
has stddev ~0.33 for these inputs, far from fp32 overflow.

Attention runs as four passes over qi-quarters (512 columns). Per
ki-chunk-PAIR one row-tile-packed scores slot computes both chunks
concurrently (chunk 2cp on PE array rows 0:63, chunk 2cp+1 on rows
64:127 — the stationaries come from [w|w]-doubled projection weights,
so rows 64:127 of qT/kT are real duplicated data, no padding). Each
chunk then gets one whole-chunk FD=512 exp — chunks alternate between
ScalarE (exact spline exp) and VectorE (Schraudolph) with their own
tiles — and one PV matmul into a 1-bank accumulator. Hard-won TRN2
scheduling rules baked in:
  - PV trails scores by FIVE chunks (pinned with add_dep_helper; sc
    bufs=7 one-bank tiles) so the in-order PE rides out exp-engine
    queueing jitter;
  - each PSUM/SBUF tile has exactly one writer and one reader engine —
    sharing a tile across engines serializes them on a false dep;
  - the PV stationaries keep full 128 partitions (vS garbage columns
    land in unread PV output rows) so LDWEIGHTS hides in the PE's
    background weight buffer;
  - the next pair's projections are woven in 8 installments through
    the current pair's attention so no engine sees a setup burst.
The kernel is exp-engine-bound (ScalarE ~86% busy, VectorE ~83%); the
PE runs warm at 2.4 GHz with ~73% occupancy.
"""

import numpy as np
import ml_dtypes

B, S, D, H = 4, 2048, 1024, 16
HD = D // H  # 64
N_CORES = 8
PAIRS_PER_CORE = (B * H) // N_CORES  # 8
KC = S // 128  # 16 ki chunks of 128
NQ = 4         # qi quarters of 512
BF16 = ml_dtypes.bfloat16

# exp split within each 1024-wide scores tile: ScalarE does [0, 512)
# into its own pT tile, VectorE does [512, 1024) into another. Separate
# tiles (and disjoint PSUM banks) keep the two exp engines fully
# independent — sharing one tile serializes them on a false WAW dep.
# Schraudolph constants for bf16-bit exp(s/8): bits = s*A + B -> int16
SCH_A = 16 * 1.4426950408889634  # 128*log2(e)/8
SCH_B = 16256.0 - 5.5 - 3.0      # bias centered so rel err ~ +-1.7%

_COMPILED = {}


def _build_nc():
    import concourse.bass as bass  # noqa: F401
    import concourse.mybir as mybir
    import concourse.tile as tile
    from concourse import bacc
    from concourse.tile_rust import add_dep_helper

    f32 = mybir.dt.float32
    bf16 = mybir.dt.bfloat16
    i16 = mybir.dt.int16

    nc = bacc.Bacc("TRN2", num_devices=N_CORES)
    xq = nc.declare_dram_parameter("xq", [PAIRS_PER_CORE, HD + 1, S], bf16, isOutput=False)
    xk = nc.declare_dram_parameter("xk", [PAIRS_PER_CORE, HD + 1, S], bf16, isOutput=False)
    xv = nc.declare_dram_parameter("xv", [PAIRS_PER_CORE, HD + 1, S], bf16, isOutput=False)
    wq = nc.declare_dram_parameter("wq", [HD + 1, 2 * HD], bf16, isOutput=False)
    wk = nc.declare_dram_parameter("wk", [HD + 1, 2 * HD], bf16, isOutput=False)
    wv = nc.declare_dram_parameter("wv", [HD + 1, HD + 1], bf16, isOutput=False)
    out = nc.declare_dram_parameter("out", [PAIRS_PER_CORE, HD + 1, S], bf16, isOutput=True)

    EXP = mybir.ActivationFunctionType.Exp
    MULT = mybir.AluOpType.mult
    ADD = mybir.AluOpType.add

    with tile.TileContext(nc) as tc:
        with (
            tc.tile_pool(name="consts", bufs=1) as consts,
            tc.tile_pool(name="ins", bufs=2) as ins_pool,
            tc.tile_pool(name="qk", bufs=2) as qk_pool,
            tc.tile_pool(name="vp", bufs=2) as v_pool,
            tc.tile_pool(name="pt", bufs=8) as pt_pool,
            tc.tile_pool(name="ob", bufs=4) as out_pool,
            tc.tile_pool(name="sc", bufs=7, space="PSUM") as sc_pool,
            tc.tile_pool(name="pv", bufs=1, space="PSUM") as pv_pool,
        ):
            w_q = consts.tile([HD + 1, 2 * HD], bf16)
            nc.sync.dma_start(out=w_q[:], in_=wq[:])
            w_k = consts.tile([HD + 1, 2 * HD], bf16)
            nc.sync.dma_start(out=w_k[:], in_=wk[:])
            w_v = consts.tile([HD + 1, HD + 1], bf16)
            nc.sync.dma_start(out=w_v[:], in_=wv[:])

            def proj_steps(j):
                """Returns ((qT, kT, vS), step_generator): DMAs pair j's
                inputs and allocates its SBUF tiles immediately; the
                generator emits the 8 projection installments (4 q/k
                blocks + 4 v groups) so the caller can spread them
                between attention chunks."""
                Xq = ins_pool.tile([HD + 1, S], bf16, tag="Xq", name="Xq")
                nc.sync.dma_start(out=Xq[:], in_=xq[j])
                Xk = ins_pool.tile([HD + 1, S], bf16, tag="Xk", name="Xk")
                nc.sync.dma_start(out=Xk[:], in_=xk[j])
                Xv = ins_pool.tile([HD + 1, S], bf16, tag="Xv", name="Xv")
                nc.sync.dma_start(out=Xv[:], in_=xv[j])

                # q^T, k^T projections -> [128, 2048] bf16 tiles. Full-128
                # stationary/rhs shapes let the PE hide LDWEIGHTS in its
                # background weight buffer (half-shape stationaries expose
                # ~90ns per weight switch). kT rows 64:128 are zeroed so the
                # padded contraction contributes nothing; qT's are zeroed so
                # no Inf/NaN garbage meets the 0*x products.
                qT = qk_pool.tile([128, S], bf16, tag="qT", name="qT")
                kT = qk_pool.tile([128, S], bf16, tag="kT", name="kT")
                vS = v_pool.tile([128, KC * 128], bf16, tag="vS", name="vS")

                def steps():
                  for n4 in range(4):
                    ps_q = sc_pool.tile([128, 512], f32, tag="sca", name="ps_q")
                    ps_k = sc_pool.tile([128, 512], f32, tag="sca", name="ps_k")
                    col = n4 * 512
                    nc.tensor.matmul(ps_q[:], w_q[:], Xq[:, col : col + 512],
                                     start=True, stop=True)
                    nc.tensor.matmul(ps_k[:], w_k[:], Xk[:, col : col + 512],
                                     start=True, stop=True)
                    nc.scalar.copy(qT[:, col : col + 512], ps_q[:])
                    if n4 == 3:
                        # re-level the exp engines: ScalarE takes one of
                        # the 4 k casts per pair off VectorE
                        nc.scalar.copy(kT[:, col : col + 512], ps_k[:])
                    else:
                        nc.vector.tensor_copy(kT[:, col : col + 512], ps_k[:])
                    yield

                  # v' projection: [2048, 65] bf16, chunk c at columns c*128
                  # (columns 65:128 of each chunk are garbage; they only
                  # feed PV output rows 65:128, which are never read)
                  vS3 = vS.rearrange("p (c d) -> p c d", d=128)
                  for g in range(4):
                    ps_v = sc_pool.tile(
                        [128, 4 * (HD + 1)], f32,
                        tag="sca", name="ps_v",
                    )
                    for c4 in range(4):
                        c = g * 4 + c4
                        nc.tensor.matmul(
                            ps_v[:, c4 * (HD + 1) : (c4 + 1) * (HD + 1)],
                            Xv[:, c * 128 : (c + 1) * 128], w_v[:],
                            start=True, stop=True,
                        )
                    nc.vector.tensor_copy(
                        vS3[:, g * 4 : (g + 1) * 4, 0 : HD + 1],
                        ps_v[:].rearrange("p (c d) -> p c d", d=HD + 1),
                    )
                    if g < 3:
                        yield

                return (qT, kT, vS), steps()

            def run_proj(pair):
                """Drain a proj_steps generator, returning the tiles."""
                state, gen = pair
                for _ in gen:
                    pass
                return state

            TRAIL = 5

            def emit_attention_pass(j, h2, qT, kT, vS, interleave=None):
                # two qi-quarter sub-passes per call: per chunk one scores
                # matmul into a 1-bank [128,512] tile (bufs=6 -> a FOUR
                # chunk trail), one whole-chunk exp on a single engine
                # (chunks alternate ScalarE / VectorE so each tile has one
                # reader and one writer), and one PV matmul into a
                # double-buffered 1-bank accumulator.
                for q4 in (2 * h2, 2 * h2 + 1):
                    base = q4 * 512
                    pv = pv_pool.tile([128, 512], f32, tag="pv", name="pv")

                    def emit_scores_exp_pair(cp):
                        # row-tiled pack: chunk 2cp on array rows 0:63,
                        # chunk 2cp+1 on rows 64:127 — both K=64 matmuls
                        # stream their N=512 columns concurrently
                        c0, c1 = 2 * cp, 2 * cp + 1
                        sca = sc_pool.tile([128, 512], f32, tag="sca", name="sca")
                        scb = sc_pool.tile([128, 512], f32, tag="sca", name="scb")
                        nc.tensor.matmul(
                            sca[:], kT[0:HD, c0 * 128 : (c0 + 1) * 128],
                            qT[0:HD, base : base + 512],
                            start=True, stop=True,
                        )
                        mm = nc.tensor.matmul(
                            scb[:], kT[HD:128, c1 * 128 : (c1 + 1) * 128],
                            qT[HD:128, base : base + 512],
                            start=True, stop=True,
                        )
                        gc1 = (base // 512) * KC + c1
                        pTa = pt_pool.tile([128, 512], bf16, tag="pTa", name="pTa")
                        nc.scalar.activation(pTa[:], sca[:], EXP, scale=0.125)
                        pTb = pt_pool.tile([128, 512], bf16, tag="pTa", name="pTb")
                        if gc1 % 32 == 15:
                            # rebalance: ScalarE takes one extra chunk per 32
                            # (VectorE carries the ob casts now)
                            nc.scalar.activation(pTb[:], scb[:], EXP, scale=0.125)
                        else:
                            nc.vector.tensor_scalar(
                                pTb[:].bitcast(i16), scb[:],
                                SCH_A, SCH_B, MULT, ADD,
                            )
                        return {c0: (pTa, mm), c1: (pTb, mm)}

                    def emit_pv(c, pT, after_mm):
                        mm = nc.tensor.matmul(
                            pv[:], vS[:, c * 128 : (c + 1) * 128], pT[:],
                            start=(c == 0), stop=(c == KC - 1),
                        )
                        if after_mm is not None:
                            add_dep_helper(
                                mm.ins, after_mm.ins, sync=False,
                                reason="pv trails scores",
                            )

                    pend = {}
                    for cp in range((TRAIL + 1) // 2):
                        pend.update(emit_scores_exp_pair(cp))
                    for c in range(KC):
                        nxt = c + TRAIL
                        if nxt < KC and nxt % 2 == 0 and nxt // 2 >= (TRAIL + 1) // 2:
                            pend.update(emit_scores_exp_pair(nxt // 2))
                        elif c % 2 == 1 and c + TRAIL + 1 < KC and (c + TRAIL + 1) // 2 >= (TRAIL + 1) // 2:
                            pend.update(emit_scores_exp_pair((c + TRAIL + 1) // 2))
                        pT_c, _ = pend.pop(c)
                        after = pend[c + TRAIL][1] if c + TRAIL in pend else None
                        emit_pv(c, pT_c, after)
                        if interleave is not None and c % 8 == 5:
                            # weave one installment of the next pair's
                            # projections between chunk groups
                            try:
                                next(interleave)
                            except StopIteration:
                                interleave = None
                    ob = out_pool.tile([HD + 1, 512], bf16, tag="ob", name="ob")
                    nc.vector.tensor_copy(ob[:], pv[0 : HD + 1, :])
                    nc.sync.dma_start(
                        out=out[j, :, base : base + 512], in_=ob[:]
                    )

            # pipeline pairs: pair j+1's DMA + projections are woven in
            # 8 installments through pair j's second attention pass, so
            # neither the PE nor the exp engines see a projection burst
            # at pair boundaries.
            state = run_proj(proj_steps(0))
            for j in range(PAIRS_PER_CORE):
                if j + 1 < PAIRS_PER_CORE:
                    nxt_state, gen = proj_steps(j + 1)
                    emit_attention_pass(j, 0, *state, interleave=gen)
                    emit_attention_pass(j, 1, *state, interleave=gen)
                    for _ in gen:
                        pass
                    state = nxt_state
                else:
                    emit_attention_pass(j, 0, *state)
                    emit_attention_pass(j, 1, *state)
    nc.finalize()
    return nc


def _get_nc():
    if "nc" not in _COMPILED:
        _COMPILED["nc"] = _build_nc()
    return _COMPILED["nc"]


def _prep_inputs(query, key_, value, Wq, bq, Wk, bk, Wv, bv):
    """Host-side repack: per (b,h) pair, [65, 2048] bf16 transposed-augmented."""
    def to_pairs(x):
        # [B, S, D] -> [B*H, HD, S] with ones row appended -> [B*H, HD+1, S]
        x = np.asarray(x, dtype=np.float32)
        x = x.reshape(B, S, H, HD).transpose(0, 2, 3, 1).reshape(B * H, HD, S)
        ones = np.ones((B * H, 1, S), dtype=np.float32)
        return np.ascontiguousarray(
            np.concatenate([x, ones], axis=1).astype(BF16)
        )

    xq_all = to_pairs(query)
    xk_all = to_pairs(key_)
    xv_all = to_pairs(value)

    Wq = np.asarray(Wq, np.float32); bq = np.asarray(bq, np.float32)
    Wk = np.asarray(Wk, np.float32); bk = np.asarray(bk, np.float32)
    Wv = np.asarray(Wv, np.float32); bv = np.asarray(bv, np.float32)
    wq_aug = np.concatenate([Wq.T, bq[None, :]], axis=0)
    wq_aug = np.concatenate([wq_aug, wq_aug], axis=1).astype(BF16)
    wk_aug = np.concatenate([Wk.T, bk[None, :]], axis=0)
    wk_aug = np.concatenate([wk_aug, wk_aug], axis=1).astype(BF16)
    wv_aug = np.zeros((HD + 1, HD + 1), np.float32)
    wv_aug[:HD, :HD] = Wv.T
    wv_aug[HD, :HD] = bv
    wv_aug[HD, HD] = 1.0
    wv_aug = wv_aug.astype(BF16)

    in_maps = []
    for i in range(N_CORES):
        sl = slice(i * PAIRS_PER_CORE, (i + 1) * PAIRS_PER_CORE)
        in_maps.append({
            "xq": np.ascontiguousarray(xq_all[sl]),
            "xk": np.ascontiguousarray(xk_all[sl]),
            "xv": np.ascontiguousarray(xv_all[sl]),
            "wq": wq_aug, "wk": wk_aug, "wv": wv_aug,
        })
    return in_maps


def _postprocess(outs):
    """outs: list of 8 arrays [8, 65, 2048] -> [B, S, D] float32."""
    full = np.concatenate(outs, axis=0).astype(np.float32)  # [64, 65, 2048]
    num = full[:, :HD, :]                # [64, 64, 2048]  (x_att^T unnormalized)
    den = full[:, HD : HD + 1, :]        # [64, 1, 2048]
    att = num / den                      # [B*H, HD, S]
    att = att.reshape(B, H, HD, S).transpose(0, 3, 1, 2).reshape(B, S, D)
    return np.ascontiguousarray(att.astype(np.float32))


def kernel(query, key_, value, Wq, bq, Wk, bk, Wv, bv, _trace=False, _res_box=None):
    import time

    from concourse.bass_utils import run_bass_kernel_spmd

    nc = _get_nc()
    in_maps = _prep_inputs(query, key_, value, Wq, bq, Wk, bk, Wv, bv)
    last_err = None
    for attempt in range(3):
        try:
            res = run_bass_kernel_spmd(
                nc, in_maps, core_ids=list(range(N_CORES)), trace=_trace
            )
            outs = [np.asarray(res.results[i]["out"]) for i in range(N_CORES)]
            break
        except Exception as e:  # transient device teardown races
            last_err = e
            time.sleep(3.0)
    else:
        raise last_err
    if _res_box is not None:
        _res_box.append(res)
    return _postprocess(outs)
